# revision 27
# baseline (speedup 1.0000x reference)
"""Trainium2 Bass kernel for nn_ChannelWiseSpatialAttentLearning.

Structure of the reference net: the only heavy compute is
    f1  = relu(conv3x3(x, w0_0) + b0_0)        # [B,256,56,56], ~59 GFLOP
    f1c = mean(f1, spatial)                    # [B,256]
Everything downstream operates on 1x1 spatial maps, so every later
"conv3x3" reduces to a center-tap matmul, and the CRF-RNN reduces to a
scalar sigmoid recurrence per sample.

Sharding: pure data parallel over batch. B=16 across 8 cores -> 2
samples/core; all params replicated.

Conv strategy per core: implicit GEMM over a zero-padded, flattened
[C, 58*58] image in SBUF. For each of the 9 taps the rhs is a shifted
contiguous column range, so each output chunk is 9 accumulating
fp8 DoubleRow matmuls (K=256 folded into one instruction via the
[Ki=128, 2, N] interleave) into one PSUM bank. fp8 weights are
pre-scaled by 16 on host; the 1/16 is folded into the relu eviction's
ACT scale. Relu + global-sum fuse into the PSUM->SBUF eviction via
accum_out. Chunks are 8 padded rows (464 cols) so legit pixels form a
clean [8,56]-stride-58 view (junk pad columns are never read/summed).
Numerics: the output sits behind a long attenuating tail ending in
sigmoids; fp8 conv inputs + bf16 tail measure ~6e-7 relative error.
"""

import sys

sys.path.insert(0, "/opt/trn_rl_repo")

import numpy as np
import ml_dtypes

B, C, H, W = 16, 256, 56, 56
CR = 64
N_CORES = 8
BPC = B // N_CORES            # samples per core
HP, WP = H + 2, W + 2         # padded 58x58
NPAD = HP * WP                # 3364
NPAD16 = 3376                 # padded to %16 elems for DoubleRow mid-dim step
P0 = WP + 1                   # 59 = first legit flat position (row1,col1)
ROWS_PER_CHUNK = 8
CHUNK = ROWS_PER_CHUNK * WP   # 464
N_CHUNKS = 7                  # 7*8 = 56 output rows
# last chunk writes only 462 cols so tap reads stay inside [0, NPAD)
CHUNK_NS = [CHUNK] * 6 + [CHUNK - 2]
W0_SCALE = 16.0               # fp8 weight pre-scale (undone in ACT eviction)

_CACHE = {}


def _build_program():
    import concourse.bacc as bacc
    import concourse.tile as tile
    from concourse import mybir

    f32 = mybir.dt.float32
    bf16 = mybir.dt.bfloat16
    f8 = mybir.dt.float8e4
    AF = mybir.ActivationFunctionType
    DR = mybir.MatmulPerfMode.DoubleRow

    nc = bacc.Bacc("TRN2", target_bir_lowering=False)

    dp = nc.declare_dram_parameter
    x_p = dp("x2", [BPC, C, H, W], f8, isOutput=False)
    w0_p = dp("w0L", [128, 9, 2, 2, 128], f8, isOutput=False)
    b00_p = dp("b00r", [128, 2], f32, isOutput=False)
    wc1_p = dp("wc1L", [128, 2, 256], bf16, isOutput=False)
    fc1_p = dp("fc1L", [128, 2, 256], bf16, isOutput=False)
    wc2_p = dp("wc2L", [128, 2, 256], bf16, isOutput=False)
    wc3_p = dp("wc3L", [128, 2, 256], bf16, isOutput=False)
    wc4_p = dp("wc4L", [128, 2, 256], bf16, isOutput=False)
    b01_p = dp("b01r", [128, 2], f32, isOutput=False)
    b02_p = dp("b02r", [128, 2], f32, isOutput=False)
    b03_p = dp("b03r", [128, 2], f32, isOutput=False)
    b04_p = dp("b04r", [128, 2], f32, isOutput=False)
    w1_p = dp("w1L", [128, 2, CR], bf16, isOutput=False)
    b1_p = dp("b1r", [CR, 1], f32, isOutput=False)
    w2_p = dp("w2L", [CR, 1], bf16, isOutput=False)
    b2_p = dp("b2r", [BPC, 1], f32, isOutput=False)
    fc2_p = dp("fc2L", [128, 2, 1], bf16, isOutput=False)
    fc2b_p = dp("fc2br", [1, 1], f32, isOutput=False)
    crf_p = dp("crfc", [BPC, 2], f32, isOutput=False)
    id2_p = dp("id2", [BPC, BPC], bf16, isOutput=False)
    out_p = dp("out", [BPC, 1], f32, isOutput=True)

    with tile.TileContext(nc) as tc:
        with (
            tc.tile_pool(name="consts", bufs=1) as consts,
            tc.tile_pool(name="frp", bufs=3) as frp,
            tc.tile_pool(name="cps", bufs=6, space="PSUM") as cps,
            tc.tile_pool(name="tps", bufs=2, space="PSUM") as tps,
        ):
            # two HWDGE issuers -> two hardware queues. Order matters: the
            # bytes that gate the first matmuls go first on each queue.
            dmaq = [nc.sync.dma_start, nc.scalar.dma_start]

            # conv weights: one whole-tile DMA, first on the sync queue
            # (starts ~1.5us before the scalar queue); x sample 0 in halves
            # on the scalar queue so early conv chunks unblock first;
            # x sample 1 follows the weights on the sync queue
            w0sb = consts.tile([128, 9, 2, 2, 128], f8, tag="w0")
            dmaq[0](out=w0sb, in_=w0_p[:])
            b00sb = consts.tile([128, 2], f32, tag="b00")
            dmaq[1](out=b00sb, in_=b00_p[:])

            # ---- x: contiguous HBM->SBUF, then pad/re-layout on-chip,
            # split into 8-row chunks across Vector and GpSimd so conv
            # chunks unblock as soon as their rows have landed ----
            xc = {}
            half = (H // 2) * W
            for s in range(BPC):
                for icb in range(2):
                    t = consts.tile([128, H * W], f8, tag=f"xc_{s}_{icb}")
                    if s == 0:
                        dmaq[1](
                            out=t[:, 0:half],
                            in_=x_p[s, icb * 128 : (icb + 1) * 128, 0 : H // 2],
                        )
                    xc[(s, icb)] = t
            for s in range(BPC):
                for icb in range(2):
                    t = xc[(s, icb)]
                    if s == 0:
                        dmaq[1](
                            out=t[:, half:],
                            in_=x_p[s, icb * 128 : (icb + 1) * 128, H // 2 :],
                        )
                    else:
                        dmaq[0](out=t, in_=x_p[s, icb * 128 : (icb + 1) * 128])
            xps = {}
            for s in range(BPC):
                t = consts.tile([128, 2, NPAD16], f8, tag=f"xp_{s}")
                for icb in range(2):
                    pl = t[:, icb, :]
                    # zero everything the relayout below does not write and
                    # the matmul taps can read: top pad row (+ row1 col0),
                    # the (col57,col0) pairs between rows, bottom pad row
                    nc.vector.memset(pl[:, 0:P0], 0.0)
                    nc.vector.memset(
                        pl[:, 115:3305].rearrange("p (k u) -> p k u", u=WP)[
                            :, :, 0:2
                        ],
                        0.0,
                    )
                    nc.vector.memset(pl[:, 3305:NPAD], 0.0)
                    dstv = pl[:, P0 : P0 + H * WP].rearrange(
                        "p (h w) -> p h w", w=WP
                    )[:, :, 0:W]
                    srcv = xc[(s, icb)].rearrange("p (h w) -> p h w", w=W)
                    eng = nc.vector if icb == 0 else nc.gpsimd
                    for c in range(N_CHUNKS):
                        r0 = ROWS_PER_CHUNK * c
                        eng.tensor_copy(
                            out=dstv[:, r0 : r0 + ROWS_PER_CHUNK, :],
                            in_=srcv[:, r0 : r0 + ROWS_PER_CHUNK, :],
                        )
                xps[s] = t

            onesb = consts.tile([BPC, 128], bf16, tag="ones")
            nc.vector.memset(onesb, 1.0)
            one1sb = consts.tile([BPC, 1], f32, tag="one1")
            nc.vector.memset(one1sb, 1.0)
            # dummy sigmoid as the FIRST activation: makes the compiler load
            # the sigmoid_and_others table (which also covers relu/identity/
            # copy) in the preamble instead of a 1.3us reload mid-tail
            actwarm = consts.tile([BPC, 1], f32, tag="actwarm")
            nc.scalar.activation(out=actwarm, in_=one1sb, func=AF.Sigmoid)
            id2sb = consts.tile([BPC, BPC], bf16, tag="id2")
            dmaq[1](out=id2sb, in_=id2_p[:])

            # ---- conv3x3 (fp8 DoubleRow, K=256 per matmul) + relu + sum ----
            partials = consts.tile([128, BPC * 2, N_CHUNKS], f32, tag="partials")
            f1sum = consts.tile([128, 2, BPC], f32, tag="f1sum")
            for s in range(BPC):
                for o in range(2):
                    for ci in range(N_CHUNKS):
                        c0 = P0 + CHUNK * ci
                        cn = CHUNK_NS[ci]
                        ps = cps.tile([128, CHUNK], f32)
                        for tap in range(9):
                            off = (tap // 3 - 1) * WP + (tap % 3 - 1)
                            nc.tensor.matmul(
                                ps[:, 0:cn],
                                w0sb[:, tap, :, o, :],
                                xps[s][:, :, c0 + off : c0 + off + cn],
                                start=(tap == 0),
                                stop=(tap == 8),
                                perf_mode=DR,
                            )
                        fr = frp.tile([128, ROWS_PER_CHUNK, W], bf16)
                        psv = ps.rearrange("p (h w) -> p h w", w=WP)[:, :, 0:W]
                        nc.scalar.activation(
                            out=fr,
                            in_=psv,
                            func=AF.Relu,
                            scale=1.0 / W0_SCALE,
                            bias=b00sb[:, o : o + 1],
                            accum_out=partials[:, o * BPC + s, ci : ci + 1],
                        )
            # partials -> f1sum -> bf16, two back-to-back DVE ops
            nc.vector.tensor_reduce(
                out=f1sum,
                in_=partials,
                axis=mybir.AxisListType.X,
                op=mybir.AluOpType.add,
            )
            f1sb = consts.tile([128, 2, BPC], bf16, tag="f1sb")
            nc.vector.tensor_copy(out=f1sb, in_=f1sum)

            # ---- tail params (emitted after conv so their DMAs don't sit
            # in front of x in the queues; they complete long before use) ----
            _ldq = [0]

            def load(pm, shape, tag, dt):
                t = consts.tile(shape, dt, tag=tag)
                dmaq[_ldq[0] % 2](out=t, in_=pm[:])
                _ldq[0] += 1
                return t

            wc1sb = load(wc1_p, [128, 2, 256], "wc1", bf16)
            fc1sb = load(fc1_p, [128, 2, 256], "fc1", bf16)
            wc2sb = load(wc2_p, [128, 2, 256], "wc2", bf16)
            wc3sb = load(wc3_p, [128, 2, 256], "wc3", bf16)
            wc4sb = load(wc4_p, [128, 2, 256], "wc4", bf16)
            b01sb = load(b01_p, [128, 2], "b01", f32)
            b02sb = load(b02_p, [128, 2], "b02", f32)
            b03sb = load(b03_p, [128, 2], "b03", f32)
            b04sb = load(b04_p, [128, 2], "b04", f32)
            w1sb = load(w1_p, [128, 2, CR], "w1", bf16)
            b1sb = load(b1_p, [CR, 1], "b1", f32)
            w2sb = load(w2_p, [CR, 1], "w2", bf16)
            b2sb = load(b2_p, [BPC, 1], "b2", f32)
            fc2sb = load(fc2_p, [128, 2, 1], "fc2", bf16)
            fc2bsb = load(fc2b_p, [1, 1], "fc2b", f32)
            crfsb = load(crf_p, [BPC, 2], "crf", f32)

            # ---- tiny tail (batch = BPC in the free dim, bf16 matmuls) ----
            def layer(dst_tag, src, wsb, bias_sb, func):
                dst = consts.tile([128, 2, BPC], bf16, tag=dst_tag)
                for o in range(2):
                    ps = tps.tile([128, BPC], f32, tag="tailps")
                    for icb in range(2):
                        nc.tensor.matmul(
                            ps,
                            wsb[:, icb, o * 128 : (o + 1) * 128],
                            src[:, icb, :],
                            start=(icb == 0),
                            stop=(icb == 1),
                        )
                    kw = {} if bias_sb is None else dict(bias=bias_sb[:, o : o + 1])
                    nc.scalar.activation(out=dst[:, o, :], in_=ps, func=func, **kw)
                return dst

            f2 = layer("f2", f1sb, wc1sb, b01sb, AF.Relu)
            vc = layer("vc", f1sb, fc1sb, None, AF.Sigmoid)
            fcm = consts.tile([128, 2, BPC], bf16, tag="fcm")
            nc.vector.tensor_mul(fcm, f2, vc)
            f3 = layer("f3", fcm, wc2sb, b02sb, AF.Relu)
            f4 = layer("f4", f3, wc3sb, b03sb, AF.Relu)

            ps64 = tps.tile([CR, BPC], f32, tag="tailps")
            for icb in range(2):
                nc.tensor.matmul(
                    ps64,
                    w1sb[:, icb, :],
                    f3[:, icb, :],
                    start=(icb == 0),
                    stop=(icb == 1),
                )
            f3s = consts.tile([CR, BPC], bf16, tag="f3s")
            nc.scalar.activation(out=f3s, in_=ps64, func=AF.Relu, bias=b1sb[:, 0:1])

            # v0s with samples on PARTITIONS (lhsT = f3s) so the whole CRF
            # recurrence can run on the ACT engine alone: per-sample values
            # become [P,1] scalars usable as ACT scale/bias operands.
            ps1 = tps.tile([BPC, 1], f32, tag="tailps")
            nc.tensor.matmul(ps1, f3s, w2sb, start=True, stop=True)
            v0s = consts.tile([BPC, 1], f32, tag="v0s")
            nc.scalar.activation(out=v0s, in_=ps1, func=AF.Relu, bias=b2sb)

            # CRF-RNN on 1x1 maps, in q-space: q_0 = sigmoid(2u);
            # q_{t+1} = sigmoid((b-a)*q_t + (2u - b)) for 5 steps, with
            # a = 0.25*(c00-c10)*s0, b = 0.25*(c01-c11)*s1.
            # crfsb rows = [b - a, -b] per sample. v_s = 1 - q_5.
            ub = consts.tile([BPC, 1], f32, tag="crf_ub")
            nc.scalar.activation(
                out=ub, in_=v0s, func=AF.Identity, scale=2.0, bias=crfsb[:, 1:2]
            )
            q = consts.tile([BPC, 1], f32, tag="crf_q0")
            nc.scalar.activation(out=q, in_=v0s, func=AF.Sigmoid, scale=2.0)
            for it in range(5):
                q2 = consts.tile([BPC, 1], f32, tag=f"crf_q{it + 1}")
                nc.scalar.activation(
                    out=q2, in_=q, func=AF.Sigmoid, scale=crfsb[:, 0:1], bias=ub
                )
                q = q2
            vs = consts.tile([BPC, 1], f32, tag="crf_vs")
            nc.scalar.activation(
                out=vs, in_=q, func=AF.Identity, scale=-1.0, bias=one1sb
            )

            # broadcast v_s across partitions: diag(vs) via DVE, then a
            # K=BPC matmul with an all-ones stationary
            vd = consts.tile([BPC, BPC], bf16, tag="crf_vd")
            nc.vector.tensor_scalar_mul(vd, id2sb, vs)
            bps = tps.tile([128, BPC], f32, tag="tailps")
            nc.tensor.matmul(bps, onesb, vd, start=True, stop=True)
            fsx = consts.tile([128, 2, BPC], bf16, tag="fsx")
            for o in range(2):
                nc.vector.tensor_mul(fsx[:, o, :], f4[:, o, :], bps)

            frr = layer("frr", fsx, wc4sb, b04sb, AF.Relu)

            psn = tps.tile([1, BPC], f32, tag="tailps")
            for icb in range(2):
                nc.tensor.matmul(
                    psn,
                    fc2sb[:, icb, :],
                    frr[:, icb, :],
                    start=(icb == 0),
                    stop=(icb == 1),
                )
            pnsb = consts.tile([1, BPC], f32, tag="pn")
            nc.scalar.activation(
                out=pnsb, in_=psn, func=AF.Sigmoid, bias=fc2bsb[:, 0:1]
            )

            dmaq[0](out=out_p[:].rearrange("b one -> one b"), in_=pnsb)

    nc.finalize()
    return nc


def _pack_shared(inputs):
    f32 = np.float32
    bf16 = ml_dtypes.bfloat16
    f8 = ml_dtypes.float8_e4m3

    w0 = np.asarray(inputs["w0_0"], f32) * W0_SCALE                # [oc, ic, 3, 3]
    # w0L[ic_in, tap, icb, ocb, oc_in] = w0[ocb*128+oc_in, icb*128+ic_in, kh, kw]
    a = w0.transpose(2, 3, 1, 0).reshape(9, 2, 128, 2, 128)        # [tap,icb,ic,ocb,oc]
    w0L = np.ascontiguousarray(a.transpose(2, 0, 1, 3, 4)).astype(f8)

    def centerT(w, scale=1.0):
        m = np.asarray(w, f32)[:, :, 1, 1].T * scale               # [ic, oc]
        ic, oc = m.shape
        return np.ascontiguousarray(
            m.reshape(ic // 128, 128, oc).transpose(1, 0, 2)
        ).astype(bf16)                                             # [128, icb, oc]

    def b2r(b):
        return np.ascontiguousarray(np.asarray(b, f32).reshape(2, 128).T)

    inv = 1.0 / (H * W)
    fc1L = np.ascontiguousarray(
        (np.asarray(inputs["fc1_w"], f32).T * inv).reshape(2, 128, 256).transpose(1, 0, 2)
    ).astype(bf16)
    fc2L = np.ascontiguousarray(
        np.asarray(inputs["fc2_w"], f32).T.reshape(2, 128, 1).transpose(1, 0, 2)
    ).astype(bf16)

    cpt = np.asarray(inputs["crf_compat"], f32)
    sw = np.asarray(inputs["crf_spatial_w"], f32)
    ca = 0.25 * (cpt[0, 0] - cpt[1, 0]) * sw[0]
    cb = 0.25 * (cpt[0, 1] - cpt[1, 1]) * sw[1]

    return {
        "w0L": w0L,
        "b00r": b2r(inputs["b0_0"]),
        "wc1L": centerT(inputs["w0_1"], inv),
        "fc1L": fc1L,
        "wc2L": centerT(inputs["w0_2"]),
        "wc3L": centerT(inputs["w0_3"]),
        "wc4L": centerT(inputs["w0_4"]),
        "b01r": b2r(inputs["b0_1"]),
        "b02r": b2r(inputs["b0_2"]),
        "b03r": b2r(inputs["b0_3"]),
        "b04r": b2r(inputs["b0_4"]),
        "w1L": centerT(inputs["w1"]),                              # [128, 2, 64]
        "b1r": np.ascontiguousarray(np.asarray(inputs["b1"], f32)[:, None]),
        "w2L": np.ascontiguousarray(
            np.asarray(inputs["w2"], f32)[:, :, 1, 1].T
        ).astype(bf16),                                            # [64, 1]
        "b2r": np.broadcast_to(
            np.asarray(inputs["b2"], f32).reshape(1, 1), (BPC, 1)
        ).copy(),
        "fc2L": fc2L,
        "fc2br": np.asarray(inputs["fc2_b"], f32).reshape(1, 1),
        "crfc": np.broadcast_to(
            np.array([[cb - ca, -cb]], f32), (BPC, 2)
        ).copy(),
        "id2": np.eye(BPC, dtype=f32).astype(bf16),
    }


def _run(inputs, trace=False):
    from concourse.bass_utils import run_bass_kernel_spmd

    if "nc" not in _CACHE:
        _CACHE["nc"] = _build_program()
    nc = _CACHE["nc"]

    shared = _pack_shared(inputs)
    x = np.asarray(inputs["x"], np.float32).astype(ml_dtypes.float8_e4m3)
    in_maps = []
    for i in range(N_CORES):
        m = dict(shared)
        m["x2"] = np.ascontiguousarray(x[i * BPC : (i + 1) * BPC])
        in_maps.append(m)

    res = run_bass_kernel_spmd(nc, in_maps, list(range(N_CORES)), trace=trace)
    out = np.concatenate(
        [res.results[i]["out"] for i in range(N_CORES)], axis=0
    ).astype(np.float32)
    return out, res


def kernel(**inputs) -> np.ndarray:
    return _run(inputs, trace=False)[0]


# revision 31
# speedup vs baseline: 1.1266x; 1.1266x over previous
"""Trainium2 Bass kernel for nn_ChannelWiseSpatialAttentLearning.

Structure of the reference net: the only heavy compute is
    f1  = relu(conv3x3(x, w0_0) + b0_0)        # [B,256,56,56], ~59 GFLOP
    f1c = mean(f1, spatial)                    # [B,256]
Everything downstream operates on 1x1 spatial maps, so every later
"conv3x3" reduces to a center-tap matmul, and the CRF-RNN reduces to a
scalar sigmoid recurrence per sample.

Sharding: pure data parallel over batch. B=16 across 8 cores -> 2
samples/core; all params replicated.

Conv strategy per core: implicit GEMM over a zero-padded, flattened
[C, 58*58] image in SBUF. For each of the 9 taps the rhs is a shifted
contiguous column range, so each output chunk is 9 accumulating
fp8 DoubleRow matmuls (K=256 folded into one instruction via the
[Ki=128, 2, N] interleave) into one PSUM bank. fp8 weights are
pre-scaled by 16 on host; the 1/16 is folded into the relu eviction's
ACT scale. Relu + global-sum fuse into the PSUM->SBUF eviction via
accum_out. Chunks are 8 padded rows (464 cols) so legit pixels form a
clean [8,56]-stride-58 view (junk pad columns are never read/summed).
Numerics: the output sits behind a long attenuating tail ending in
sigmoids; fp8 conv inputs + bf16 tail measure ~6e-7 relative error.
"""

import sys

sys.path.insert(0, "/opt/trn_rl_repo")

import numpy as np
import ml_dtypes

B, C, H, W = 16, 256, 56, 56
CR = 64
N_CORES = 8
BPC = B // N_CORES            # samples per core
HP, WP = H + 2, W + 2         # padded 58x58
NPAD16 = 3376                 # plane size, %16 for the DoubleRow mid-dim step
# first legit pixel lives at byte 60 (not 59): even offset so the on-chip
# relayout can run as uint16 moves (fp8 elementwise is ~4x slower on DVE).
# Taps are relative shifts, so sliding the whole plane by +1 is transparent.
B0 = 60
# reads span [B0-59, B0+55*58+55+59] = [1, 3364] -- inside [0, 3376)
ROWS_PER_CHUNK = 8
CHUNK = ROWS_PER_CHUNK * WP   # 464
N_CHUNKS = 7                  # 7*8 = 56 output rows
# last chunk writes only 462 cols so tap reads stay inside [0, NPAD)
CHUNK_NS = [CHUNK] * 6 + [CHUNK - 2]
W0_SCALE = 16.0               # fp8 weight pre-scale (undone in ACT eviction)

_CACHE = {}


def _build_program():
    import concourse.bacc as bacc
    import concourse.tile as tile
    from concourse import mybir

    f32 = mybir.dt.float32
    bf16 = mybir.dt.bfloat16
    f8 = mybir.dt.float8e4
    AF = mybir.ActivationFunctionType
    DR = mybir.MatmulPerfMode.DoubleRow

    nc = bacc.Bacc("TRN2", target_bir_lowering=False)

    dp = nc.declare_dram_parameter
    x_p = dp("x2", [BPC, C, H, W], f8, isOutput=False)
    w0_p = dp("w0L", [128, 9, 2, 2, 128], f8, isOutput=False)
    b00_p = dp("b00r", [128, 2], f32, isOutput=False)
    wc1_p = dp("wc1L", [128, 2, 256], bf16, isOutput=False)
    fc1_p = dp("fc1L", [128, 2, 256], bf16, isOutput=False)
    wc2_p = dp("wc2L", [128, 2, 256], bf16, isOutput=False)
    wc3_p = dp("wc3L", [128, 2, 256], bf16, isOutput=False)
    wc4_p = dp("wc4L", [128, 2, 256], bf16, isOutput=False)
    b01_p = dp("b01r", [128, 2], f32, isOutput=False)
    b02_p = dp("b02r", [128, 2], f32, isOutput=False)
    b03_p = dp("b03r", [128, 2], f32, isOutput=False)
    b04_p = dp("b04r", [128, 2], f32, isOutput=False)
    w1_p = dp("w1L", [128, 2, CR], bf16, isOutput=False)
    b1_p = dp("b1r", [CR, 1], f32, isOutput=False)
    w2_p = dp("w2L", [CR, 1], bf16, isOutput=False)
    b2_p = dp("b2r", [BPC, 1], f32, isOutput=False)
    fc2_p = dp("fc2L", [128, 2, 1], bf16, isOutput=False)
    fc2b_p = dp("fc2br", [1, 1], f32, isOutput=False)
    crf_p = dp("crfc", [BPC, 2], f32, isOutput=False)
    id2_p = dp("id2", [BPC, BPC], bf16, isOutput=False)
    out_p = dp("out", [BPC, 1], f32, isOutput=True)

    with tile.TileContext(nc) as tc:
        with (
            tc.tile_pool(name="consts", bufs=1) as consts,
            tc.tile_pool(name="frp", bufs=3) as frp,
            tc.tile_pool(name="cps", bufs=6, space="PSUM") as cps,
            tc.tile_pool(name="tps", bufs=2, space="PSUM") as tps,
        ):
            # two HWDGE issuers -> two hardware queues. Order matters: the
            # bytes that gate the first matmuls go first on each queue.
            dmaq = [nc.sync.dma_start, nc.scalar.dma_start]

            # x(s0,icb0) + conv weights on the sync queue (starts ~1.5us
            # before the scalar queue); x(s0,icb1) first on the scalar queue
            w0sb = consts.tile([128, 9, 2, 2, 128], f8, tag="w0")
            xc = {}
            for s in range(BPC):
                for icb in range(2):
                    t = consts.tile([128, H * W], f8, tag=f"xc_{s}_{icb}")
                    xc[(s, icb)] = t

            def ldx(s, icb, q):
                dmaq[q](
                    out=xc[(s, icb)], in_=x_p[s, icb * 128 : (icb + 1) * 128]
                )

            ldx(0, 0, 0)
            ldx(0, 1, 1)
            dmaq[0](out=w0sb, in_=w0_p[:])
            b00sb = consts.tile([128, 2], f32, tag="b00")
            dmaq[1](out=b00sb, in_=b00_p[:])
            ldx(1, 0, 0)
            ldx(1, 1, 1)

            # pad/re-layout on-chip as uint16 moves (even byte offsets by
            # construction of B0), split into 8-row chunks; Vector owns the
            # critical sample 0, GpSimd does sample 1 in parallel
            u16 = mybir.dt.uint16
            xps = {}
            for s in range(BPC):
                t = consts.tile([128, 2, NPAD16], f8, tag=f"xp_{s}")
                xps[s] = t
            for s in range(BPC):
                t = xps[s]
                eng = nc.vector if s == 0 else nc.gpsimd
                dstv = {}
                srcv = {}
                for icb in range(2):
                    pl = t[:, icb, :]
                    # zero everything the relayout below does not write and
                    # the matmul taps can read: head pad, the two junk cols
                    # between rows, tail pad
                    nc.vector.memset(pl[:, 0:B0], 0.0)
                    nc.vector.memset(
                        pl[:, 116:3306].rearrange("p (k u) -> p k u", u=WP)[
                            :, :, 0:2
                        ],
                        0.0,
                    )
                    nc.vector.memset(pl[:, 3306:NPAD16], 0.0)
                    dstv[icb] = pl.bitcast(u16)[:, B0 // 2 : B0 // 2 + 29 * H] \
                        .rearrange("p (h w) -> p h w", w=29)[:, :, 0:28]
                    srcv[icb] = xc[(s, icb)].bitcast(u16).rearrange(
                        "p (h w) -> p h w", w=28
                    )
                for c in range(N_CHUNKS):
                    r0 = ROWS_PER_CHUNK * c
                    for icb in range(2):
                        eng.tensor_copy(
                            out=dstv[icb][:, r0 : r0 + ROWS_PER_CHUNK, :],
                            in_=srcv[icb][:, r0 : r0 + ROWS_PER_CHUNK, :],
                        )

            onesb = consts.tile([BPC, 128], bf16, tag="ones")
            nc.vector.memset(onesb, 1.0)
            one1sb = consts.tile([BPC, 1], f32, tag="one1")
            nc.vector.memset(one1sb, 1.0)
            # dummy sigmoid as the FIRST activation: makes the compiler load
            # the sigmoid_and_others table (which also covers relu/identity/
            # copy) in the preamble instead of a 1.3us reload mid-tail
            actwarm = consts.tile([BPC, 1], f32, tag="actwarm")
            nc.scalar.activation(out=actwarm, in_=one1sb, func=AF.Sigmoid)
            id2sb = consts.tile([BPC, BPC], bf16, tag="id2")
            dmaq[1](out=id2sb, in_=id2_p[:])

            # ---- conv3x3 (fp8 DoubleRow, K=256 per matmul) + relu + sum ----
            partials = consts.tile([128, BPC * 2, N_CHUNKS], f32, tag="partials")
            f1sum = consts.tile([128, 2, BPC], f32, tag="f1sum")
            for s in range(BPC):
                for o in range(2):
                    for ci in range(N_CHUNKS):
                        c0 = B0 + CHUNK * ci
                        cn = CHUNK_NS[ci]
                        ps = cps.tile([128, CHUNK], f32)
                        for tap in range(9):
                            off = (tap // 3 - 1) * WP + (tap % 3 - 1)
                            nc.tensor.matmul(
                                ps[:, 0:cn],
                                w0sb[:, tap, :, o, :],
                                xps[s][:, :, c0 + off : c0 + off + cn],
                                start=(tap == 0),
                                stop=(tap == 8),
                                perf_mode=DR,
                            )
                        fr = frp.tile([128, ROWS_PER_CHUNK, W], bf16)
                        psv = ps.rearrange("p (h w) -> p h w", w=WP)[:, :, 0:W]
                        nc.scalar.activation(
                            out=fr,
                            in_=psv,
                            func=AF.Relu,
                            scale=1.0 / W0_SCALE,
                            bias=b00sb[:, o : o + 1],
                            accum_out=partials[:, o * BPC + s, ci : ci + 1],
                        )
            # partials -> f1sum -> bf16, two back-to-back DVE ops
            nc.vector.tensor_reduce(
                out=f1sum,
                in_=partials,
                axis=mybir.AxisListType.X,
                op=mybir.AluOpType.add,
            )
            f1sb = consts.tile([128, 2, BPC], bf16, tag="f1sb")
            nc.vector.tensor_copy(out=f1sb, in_=f1sum)

            # ---- tail params (emitted after conv so their DMAs don't sit
            # in front of x in the queues; they complete long before use) ----
            _ldq = [0]

            def load(pm, shape, tag, dt):
                t = consts.tile(shape, dt, tag=tag)
                dmaq[_ldq[0] % 2](out=t, in_=pm[:])
                _ldq[0] += 1
                return t

            wc1sb = load(wc1_p, [128, 2, 256], "wc1", bf16)
            fc1sb = load(fc1_p, [128, 2, 256], "fc1", bf16)
            wc2sb = load(wc2_p, [128, 2, 256], "wc2", bf16)
            wc3sb = load(wc3_p, [128, 2, 256], "wc3", bf16)
            wc4sb = load(wc4_p, [128, 2, 256], "wc4", bf16)
            b01sb = load(b01_p, [128, 2], "b01", f32)
            b02sb = load(b02_p, [128, 2], "b02", f32)
            b03sb = load(b03_p, [128, 2], "b03", f32)
            b04sb = load(b04_p, [128, 2], "b04", f32)
            w1sb = load(w1_p, [128, 2, CR], "w1", bf16)
            b1sb = load(b1_p, [CR, 1], "b1", f32)
            w2sb = load(w2_p, [CR, 1], "w2", bf16)
            b2sb = load(b2_p, [BPC, 1], "b2", f32)
            fc2sb = load(fc2_p, [128, 2, 1], "fc2", bf16)
            fc2bsb = load(fc2b_p, [1, 1], "fc2b", f32)
            crfsb = load(crf_p, [BPC, 2], "crf", f32)

            # ---- tiny tail (batch = BPC in the free dim, bf16 matmuls) ----
            def layer(dst_tag, src, wsb, bias_sb, func):
                dst = consts.tile([128, 2, BPC], bf16, tag=dst_tag)
                for o in range(2):
                    ps = tps.tile([128, BPC], f32, tag="tailps")
                    for icb in range(2):
                        nc.tensor.matmul(
                            ps,
                            wsb[:, icb, o * 128 : (o + 1) * 128],
                            src[:, icb, :],
                            start=(icb == 0),
                            stop=(icb == 1),
                        )
                    kw = {} if bias_sb is None else dict(bias=bias_sb[:, o : o + 1])
                    nc.scalar.activation(out=dst[:, o, :], in_=ps, func=func, **kw)
                return dst

            f2 = layer("f2", f1sb, wc1sb, b01sb, AF.Relu)
            vc = layer("vc", f1sb, fc1sb, None, AF.Sigmoid)
            fcm = consts.tile([128, 2, BPC], bf16, tag="fcm")
            nc.vector.tensor_mul(fcm, f2, vc)
            f3 = layer("f3", fcm, wc2sb, b02sb, AF.Relu)
            f4 = layer("f4", f3, wc3sb, b03sb, AF.Relu)

            ps64 = tps.tile([CR, BPC], f32, tag="tailps")
            for icb in range(2):
                nc.tensor.matmul(
                    ps64,
                    w1sb[:, icb, :],
                    f3[:, icb, :],
                    start=(icb == 0),
                    stop=(icb == 1),
                )
            f3s = consts.tile([CR, BPC], bf16, tag="f3s")
            nc.scalar.activation(out=f3s, in_=ps64, func=AF.Relu, bias=b1sb[:, 0:1])

            # v0s with samples on PARTITIONS (lhsT = f3s) so the whole CRF
            # recurrence can run on the ACT engine alone: per-sample values
            # become [P,1] scalars usable as ACT scale/bias operands.
            ps1 = tps.tile([BPC, 1], f32, tag="tailps")
            nc.tensor.matmul(ps1, f3s, w2sb, start=True, stop=True)
            v0s = consts.tile([BPC, 1], f32, tag="v0s")
            nc.scalar.activation(out=v0s, in_=ps1, func=AF.Relu, bias=b2sb)

            # CRF-RNN on 1x1 maps, in q-space: q_0 = sigmoid(2u);
            # q_{t+1} = sigmoid((b-a)*q_t + (2u - b)) for 5 steps, with
            # a = 0.25*(c00-c10)*s0, b = 0.25*(c01-c11)*s1.
            # crfsb rows = [b - a, -b] per sample. v_s = 1 - q_5.
            ub = consts.tile([BPC, 1], f32, tag="crf_ub")
            nc.scalar.activation(
                out=ub, in_=v0s, func=AF.Identity, scale=2.0, bias=crfsb[:, 1:2]
            )
            q = consts.tile([BPC, 1], f32, tag="crf_q0")
            nc.scalar.activation(out=q, in_=v0s, func=AF.Sigmoid, scale=2.0)
            for it in range(5):
                q2 = consts.tile([BPC, 1], f32, tag=f"crf_q{it + 1}")
                nc.scalar.activation(
                    out=q2, in_=q, func=AF.Sigmoid, scale=crfsb[:, 0:1], bias=ub
                )
                q = q2
            vs = consts.tile([BPC, 1], f32, tag="crf_vs")
            nc.scalar.activation(
                out=vs, in_=q, func=AF.Identity, scale=-1.0, bias=one1sb
            )

            # broadcast v_s across partitions: diag(vs) via DVE, then a
            # K=BPC matmul with an all-ones stationary
            vd = consts.tile([BPC, BPC], bf16, tag="crf_vd")
            nc.vector.tensor_scalar_mul(vd, id2sb, vs)
            bps = tps.tile([128, BPC], f32, tag="tailps")
            nc.tensor.matmul(bps, onesb, vd, start=True, stop=True)
            fsx = consts.tile([128, 2, BPC], bf16, tag="fsx")
            for o in range(2):
                nc.vector.tensor_mul(fsx[:, o, :], f4[:, o, :], bps)

            frr = layer("frr", fsx, wc4sb, b04sb, AF.Relu)

            psn = tps.tile([1, BPC], f32, tag="tailps")
            for icb in range(2):
                nc.tensor.matmul(
                    psn,
                    fc2sb[:, icb, :],
                    frr[:, icb, :],
                    start=(icb == 0),
                    stop=(icb == 1),
                )
            pnsb = consts.tile([1, BPC], f32, tag="pn")
            nc.scalar.activation(
                out=pnsb, in_=psn, func=AF.Sigmoid, bias=fc2bsb[:, 0:1]
            )

            dmaq[0](out=out_p[:].rearrange("b one -> one b"), in_=pnsb)

    nc.finalize()
    return nc


def _pack_shared(inputs):
    f32 = np.float32
    bf16 = ml_dtypes.bfloat16
    f8 = ml_dtypes.float8_e4m3

    w0 = np.asarray(inputs["w0_0"], f32) * W0_SCALE                # [oc, ic, 3, 3]
    # w0L[ic_in, tap, icb, ocb, oc_in] = w0[ocb*128+oc_in, icb*128+ic_in, kh, kw]
    a = w0.transpose(2, 3, 1, 0).reshape(9, 2, 128, 2, 128)        # [tap,icb,ic,ocb,oc]
    w0L = np.ascontiguousarray(a.transpose(2, 0, 1, 3, 4)).astype(f8)

    def centerT(w, scale=1.0):
        m = np.asarray(w, f32)[:, :, 1, 1].T * scale               # [ic, oc]
        ic, oc = m.shape
        return np.ascontiguousarray(
            m.reshape(ic // 128, 128, oc).transpose(1, 0, 2)
        ).astype(bf16)                                             # [128, icb, oc]

    def b2r(b):
        return np.ascontiguousarray(np.asarray(b, f32).reshape(2, 128).T)

    inv = 1.0 / (H * W)
    fc1L = np.ascontiguousarray(
        (np.asarray(inputs["fc1_w"], f32).T * inv).reshape(2, 128, 256).transpose(1, 0, 2)
    ).astype(bf16)
    fc2L = np.ascontiguousarray(
        np.asarray(inputs["fc2_w"], f32).T.reshape(2, 128, 1).transpose(1, 0, 2)
    ).astype(bf16)

    cpt = np.asarray(inputs["crf_compat"], f32)
    sw = np.asarray(inputs["crf_spatial_w"], f32)
    ca = 0.25 * (cpt[0, 0] - cpt[1, 0]) * sw[0]
    cb = 0.25 * (cpt[0, 1] - cpt[1, 1]) * sw[1]

    return {
        "w0L": w0L,
        "b00r": b2r(inputs["b0_0"]),
        "wc1L": centerT(inputs["w0_1"], inv),
        "fc1L": fc1L,
        "wc2L": centerT(inputs["w0_2"]),
        "wc3L": centerT(inputs["w0_3"]),
        "wc4L": centerT(inputs["w0_4"]),
        "b01r": b2r(inputs["b0_1"]),
        "b02r": b2r(inputs["b0_2"]),
        "b03r": b2r(inputs["b0_3"]),
        "b04r": b2r(inputs["b0_4"]),
        "w1L": centerT(inputs["w1"]),                              # [128, 2, 64]
        "b1r": np.ascontiguousarray(np.asarray(inputs["b1"], f32)[:, None]),
        "w2L": np.ascontiguousarray(
            np.asarray(inputs["w2"], f32)[:, :, 1, 1].T
        ).astype(bf16),                                            # [64, 1]
        "b2r": np.broadcast_to(
            np.asarray(inputs["b2"], f32).reshape(1, 1), (BPC, 1)
        ).copy(),
        "fc2L": fc2L,
        "fc2br": np.asarray(inputs["fc2_b"], f32).reshape(1, 1),
        "crfc": np.broadcast_to(
            np.array([[cb - ca, -cb]], f32), (BPC, 2)
        ).copy(),
        "id2": np.eye(BPC, dtype=f32).astype(bf16),
    }


def _run(inputs, trace=False):
    from concourse.bass_utils import run_bass_kernel_spmd

    if "nc" not in _CACHE:
        _CACHE["nc"] = _build_program()
    nc = _CACHE["nc"]

    shared = _pack_shared(inputs)
    x = np.asarray(inputs["x"], np.float32).astype(ml_dtypes.float8_e4m3)
    in_maps = []
    for i in range(N_CORES):
        m = dict(shared)
        m["x2"] = np.ascontiguousarray(x[i * BPC : (i + 1) * BPC])
        in_maps.append(m)

    res = run_bass_kernel_spmd(nc, in_maps, list(range(N_CORES)), trace=trace)
    out = np.concatenate(
        [res.results[i]["out"] for i in range(N_CORES)], axis=0
    ).astype(np.float32)
    return out, res


def kernel(**inputs) -> np.ndarray:
    return _run(inputs, trace=False)[0]
